# revision 48
# baseline (speedup 1.0000x reference)
"""Trainium2 Bass kernel for nn_Attention (GQA + RoPE + softmax-n + causal).

Full inputs -> shard DP2(batch) x TP4(heads) across 8 cores -> gather+sum.

Per-core device program (all matmuls fp32r, PSUM fp32):
  phase 1: Q^T/K^T/V^T = w.T @ x^T   (x^T streamed in 512-col chunks)
           RoPE on Q^T/K^T via sign-folded tables + DMA partition half-swap
           V^T transposed back to natural V via PE transpose
  phase 2: per q-chunk (512 cols), per head:
           scores^T[k,q] = K^T.T @ Q^T  (causal: N-sliced bands)
           E = exp(scores^T)  (softmax-n: no max subtraction; scores ~N(0,0.8))
           diag 128x128 blocks masked by multiplying a triangle mask
           denom[1,q] = ones.T @ E (+1 phantom logit), accumulated in PSUM
           out^T[hd,q] += V.T @ E ;  out^T *= broadcast(1/denom)
           then output projection for this q-chunk: out += oc.T @ wo_shard

Host: out[b] = sum over 4 TP shards of out_partial.
"""
import sys
import numpy as np

sys.path.insert(0, "/opt/trn_rl_repo")

import concourse.bass as bass
import concourse.bacc as bacc
import concourse.mybir as mybir
import concourse.tile as tile
from concourse import bass_utils
from concourse._compat import with_exitstack

F32 = mybir.dt.float32
F32R = mybir.dt.float32r
EXP = mybir.ActivationFunctionType.Exp
COPY = mybir.ActivationFunctionType.Copy

B, S, D = 2, 2048, 2048
N_HEADS, N_KV_HEADS, HD = 16, 8, 128
TP = 4                      # tensor-parallel ways (x DP2 over batch = 8 cores)
QF = 4 * HD                 # per-core q feature cols   (512)
KF = 2 * HD                 # per-core k/v feature cols (256)
NQT = S // 128              # 16 seq tiles
NQC = S // 512              # 4  q-chunks
ND = D // 128               # 16 contraction tiles
NSC = S // 512              # 4  x^T stream chunks

_CACHE = {}

# matmul dtype config: "f32r" or "bf16" per stage
import os
CFG = {"proj": os.environ.get("K_PROJ", "bf16"),
       "attn": os.environ.get("K_ATTN", "f32r"),
       "wo": os.environ.get("K_WO", "f32r")}


def _dt(stage):
    return F32R if CFG[stage] == "f32r" else mybir.dt.bfloat16


def _npdt(stage):
    import ml_dtypes
    return np.float32 if CFG[stage] == "f32r" else ml_dtypes.bfloat16


def _build(bench_reps=None):
    nc = bacc.Bacc("TRN2", target_bir_lowering=False, debug=False)

    PJ, AT, WD = _dt("proj"), _dt("attn"), _dt("wo")
    names = [("xT", [D, S], PJ), ("wq", [D, QF], PJ), ("wk", [D, KF], PJ),
             ("wv", [D, KF], PJ), ("wo", [QF, D], WD),
             ("c2", [128, S], F32), ("g", [128, S], F32),
             ("tri", [128, 128], AT), ("tri2", [128, 256], AT),
             ("ones128", [128, 1], AT), ("idn", [128, 128], AT)]
    kind = "Internal" if bench_reps else "ExternalInput"
    io = {n: nc.dram_tensor(n, sh, dt, kind=kind) for n, sh, dt in names}
    if bench_reps:
        io["dummy"] = nc.dram_tensor("bench_in", [128, 1], F32,
                                     kind="ExternalInput")
    # bf16 out halves the 16MB/core output traffic; host upcasts+sums
    io["out"] = nc.dram_tensor("out", [S, D], mybir.dt.bfloat16,
                               kind="ExternalOutput")
    if os.environ.get("K_DEBUG"):
        io["dbg_qk"] = nc.dram_tensor("dbg_qk", [6 * 128, S], F32,
                                      kind="ExternalOutput")
        io["dbg_v"] = nc.dram_tensor("dbg_v", [128, KF], F32,
                                     kind="ExternalOutput")

    with tile.TileContext(nc) as tc:
        if bench_reps:
            # fill internal DRAM inputs with benign constants (avoid
            # garbage -> denormal/NaN timing artifacts)
            with tc.tile_pool(name="fillp", bufs=1) as fp:
                f3t = fp.tile([128, 2048], F32, tag="fill32")
                fbt = fp.tile([128, 2048], mybir.dt.bfloat16, tag="fillb")
                nc.gpsimd.memset(f3t[:], 0.001)
                nc.gpsimd.memset(fbt[:], 0.001)
                for n, sh, dt in names:
                    r, c = sh
                    for r0 in range(0, r, 128):
                        rr = min(128, r - r0)
                        for c0 in range(0, c, 2048):
                            cc = min(2048, c - c0)
                            if dt == mybir.dt.bfloat16:
                                srcap = fbt[:rr, :cc]
                            elif dt == F32R:
                                srcap = f3t[:rr, :cc].bitcast(F32R)
                            else:
                                srcap = f3t[:rr, :cc]
                            nc.sync.dma_start(io[n][r0:r0 + rr, c0:c0 + cc],
                                              srcap)
        if bench_reps and bench_reps > 1:
            with tc.For_i(0, bench_reps, 1):
                _emit(tc, nc, io)
        else:
            _emit(tc, nc, io)
    nc.compile()
    return nc


@with_exitstack
def _emit(ctx, tc, nc, io):
    ts = bass.ts
    PJ, AT, WD = _dt("proj"), _dt("attn"), _dt("wo")
    persist = ctx.enter_context(tc.tile_pool(name="persist", bufs=1))

    # ---- persistent SBUF tensors (live whole kernel) ----
    tri = persist.tile([128, 128], AT, tag="tri")
    tri2 = persist.tile([128, 256], AT, tag="tri2")
    ones128 = persist.tile([128, 1], AT, tag="ones128")

    # rotated Q^T/K^T: 6 head tiles [128, S]; V natural: 16 tiles [128, KF]
    qkT = [persist.tile([128, S], AT, tag=f"qkT{f}", name=f"qkT{f}")
           for f in range(6)]
    vnat = [persist.tile([128, KF], AT, tag=f"vnat{st}", name=f"vnat{st}")
            for st in range(NQT)]
    # ================= phase 1: projections + rope + V transpose ==========
    # DMA instruction count is the scarce resource (~630ns HWDGE + ~650ns
    # SEQ per instruction regardless of size), so weights/x^T load as wide
    # merged tiles via rearranged access patterns: wq in 4 DMAs, wk/wv in 2
    # each, x^T in 4 per 512-col chunk, c2/g in 1 each (full S).
    with tc.tile_pool(name="wp", bufs=1) as wp, \
         tc.tile_pool(name="xtp", bufs=2) as xtp, \
         tc.tile_pool(name="rope", bufs=3) as ropep, \
         tc.tile_pool(name="vsb", bufs=2) as vsbp, \
         tc.tile_pool(name="p1ps", bufs=5, space="PSUM") as p1ps, \
         tc.tile_pool(name="vtps", bufs=3, space="PSUM") as vtps:
        idn = wp.tile([128, 128], AT, tag="idn")
        wq_all = wp.tile([128, ND * QF], PJ, tag="wq")      # [128, 8192]
        wk_all = wp.tile([128, ND * KF], PJ, tag="wk")      # [128, 4096]
        wv_all = wp.tile([128, ND * KF], PJ, tag="wv")
        c2_all = wp.tile([128, S], F32, tag="c2")
        g_all = wp.tile([128, S], F32, tag="g")

        def load_xt(sc, t):
            for gq in range(4):
                dst = t[:, gq * 2048:(gq + 1) * 2048].rearrange(
                    "p (d c) -> p d c", d=4)
                src = io["xT"][ts(gq, 512), ts(sc, 512)].rearrange(
                    "(d p) c -> p d c", d=4)
                nc.scalar.dma_start(dst, src)
            return t

        # V chains run first in the f-loop (smallest weight gate: wv 1MB vs
        # wq 2MB), so interleave wv groups with x^T chunk 0; wq lands while
        # the V chains execute.
        xt_tiles = [None] * NSC
        t0 = xtp.tile([128, ND * 512], PJ, tag="xt", name="xt0")

        def wv_grp(d0, gd):
            dst = wv_all[:, d0 * 256:(d0 + gd) * 256].rearrange(
                "p (d c) -> p d c", d=gd)
            src = io["wv"][d0 * 128:(d0 + gd) * 128, :].rearrange(
                "(d p) c -> p d c", d=gd)
            nc.sync.dma_start(dst, src)

        def xt0_grp(d0, gd):
            dstx = t0[:, d0 * 512:(d0 + gd) * 512].rearrange(
                "p (d c) -> p d c", d=gd)
            srcx = io["xT"][d0 * 128:(d0 + gd) * 128, 0:512].rearrange(
                "(d p) c -> p d c", d=gd)
            nc.scalar.dma_start(dstx, srcx)

        wv_grp(0, 2)
        xt0_grp(0, 2)
        wv_grp(2, 2)
        xt0_grp(2, 2)
        wv_grp(4, 4)
        xt0_grp(4, 4)
        wv_grp(8, 8)
        xt0_grp(8, 4)
        xt0_grp(12, 4)
        nc.sync.dma_start(idn[:], io["idn"][:])
        for gq in range(4):
            dst = wq_all[:, gq * 2048:(gq + 1) * 2048].rearrange(
                "p (d c) -> p d c", d=4)
            src = io["wq"][ts(gq, 512), :].rearrange("(d p) c -> p d c", d=4)
            nc.sync.dma_start(dst, src)
        xt_tiles[0] = t0
        add_pend = []
        # c2/g transfers after the startup-critical wv/xt0 stream
        nc.gpsimd.dma_start(c2_all[:], io["c2"][:])
        nc.gpsimd.dma_start(g_all[:], io["g"][:])
        for gk in range(2):
            dst = wk_all[:, gk * 2048:(gk + 1) * 2048].rearrange(
                "p (d c) -> p d c", d=8)
            src = io["wk"][ts(gk, 1024), :].rearrange("(d p) c -> p d c", d=8)
            nc.sync.dma_start(dst, src)
        for name, t in [("tri", tri), ("tri2", tri2), ("ones128", ones128)]:
            nc.sync.dma_start(t[:], io[name][:])

        for sc in range(NSC):                    # 512-wide x^T chunks
            cs = ts(sc, 512)
            xt = xt_tiles[sc]
            if sc + 1 < NSC:                     # prefetch next chunk
                xt_tiles[sc + 1] = load_xt(
                    sc + 1, xtp.tile([128, ND * 512], PJ, tag="xt",
                                     name=f"xt{sc + 1}"))
            vts = []
            # f: 0..3 q-heads, 4..5 k-heads, 6..7 v-heads; V first (their
            # weights land first at startup), k last (their rope adds are
            # deferred anyway)
            for f in (6, 7, 0, 1, 2, 3, 4, 5):
                if f < 4:
                    wall, wstride, fo = wq_all, QF, f * 128
                elif f < 6:
                    wall, wstride, fo = wk_all, KF, (f - 4) * 128
                else:
                    wall, wstride, fo = wv_all, KF, (f - 6) * 128
                ps = p1ps.tile([128, 512], F32, tag="proj")
                for d in range(ND):
                    nc.tensor.matmul(
                        ps[:],
                        wall[:, d * wstride + fo:d * wstride + fo + 128],
                        xt[:, ts(d, 512)],
                        start=(d == 0), stop=(d == ND - 1))
                if f < 6:
                    # rope: rot = ps*c2 + halfswap(ps*g)
                    a = ropep.tile([128, 512], F32, tag="ropeA")
                    b = ropep.tile([128, 512], F32, tag="ropeB")
                    bsw = ropep.tile([128, 512], F32, tag="ropeBsw")
                    nc.vector.tensor_mul(a[:], ps[:], c2_all[:, cs])
                    nc.vector.tensor_mul(b[:], ps[:], g_all[:, cs])
                    nc.gpsimd.dma_start(bsw[0:64, :], b[64:128, :])
                    nc.gpsimd.dma_start(bsw[64:128, :], b[0:64, :])
                    if f < 4:
                        nc.vector.tensor_add(qkT[f][:, cs], a[:], bsw[:])
                    else:
                        # defer the k-head adds past the V copies: the add
                        # waits on the swap DMA, and queuing it ahead of the
                        # V copies on DVE stalls the PE transposes at each
                        # sc boundary (the adds aren't needed until phase 2)
                        add_pend.append((f, a, bsw, cs))
                else:
                    # V^T -> copy to SBUF; emitted before the deferred k-head
                    # adds so the PE transposes never queue behind the
                    # swap-DMA-gated rope adds on DVE
                    vt = vsbp.tile([128, 512], AT, tag="vT")
                    nc.vector.tensor_copy(vt[:], ps[:])
                    vts.append((f, vt))
            for f, vt in vts:
                for sub in range(4):
                    st = sc * 4 + sub
                    tp = vtps.tile([128, 128], AT, tag="vtp")
                    nc.tensor.transpose(tp[:], vt[:, ts(sub, 128)], idn[:])
                    nc.vector.tensor_copy(
                        vnat[st][:, (f - 6) * 128:(f - 5) * 128], tp[:])
            for fp, ap_, bswp, csp in add_pend:
                nc.vector.tensor_add(qkT[fp][:, csp], ap_[:], bswp[:])
            add_pend = []

    # ============ phase 2: attention + fused output projection ============
    # Software-pipelined: PE stream = scores(i), scores(i+1), then
    # AV/den(i-LAG) interleaved, so the ACT exp latency never stalls PE.
    # The wo projection for q-chunk qc-1 is interleaved into qc's attention
    # so the recip/broadcast latency at each chunk boundary is hidden.
    with tc.tile_pool(name="ep", bufs=8) as ep, \
         tc.tile_pool(name="ocp", bufs=2) as ocp, \
         tc.tile_pool(name="fin", bufs=2) as finp, \
         tc.tile_pool(name="osb", bufs=3) as osbp, \
         tc.tile_pool(name="scps", bufs=3, space="PSUM") as scps, \
         tc.tile_pool(name="outps", bufs=2, space="PSUM") as outps, \
         tc.tile_pool(name="denps", bufs=1, space="PSUM") as denps, \
         tc.tile_pool(name="w3ps", bufs=2, space="PSUM") as w3ps, \
         tc.tile_pool(name="wop", bufs=1) as wop:
        wo_sb = [wop.tile([128, D], WD, tag=f"wo{hf}", name=f"wo{hf}")
                 for hf in range(4)]
        for hf in range(4):
            nc.sync.dma_start(wo_sb[hf][:], io["wo"][ts(hf, 128), :])

        LAG = 4
        pend = {}          # (h, kt) -> (e tile, off)
        hstate = {}        # h -> (out_ps, den_ps)
        oc_cur = [None] * 4
        oc_prev = None

        def emit_scores(qc, h, kt):
            qs = qc * 512
            gkv = h // 2
            qT, kT = qkT[h], qkT[4 + gkv]
            off = max(0, 128 * kt - qs)
            diag = kt >= 4 * qc
            moff = off
            if off == 384:
                off = 256        # keep N>=256 (fp32r full rate)
            sc_ps = scps.tile([128, 512], F32, tag="sc")
            nc.tensor.matmul(sc_ps[:, off:], kT[:, ts(kt, 128)],
                             qT[:, qs + off:qs + 512],
                             start=True, stop=True)
            e = ep.tile([128, 512], AT, tag="e")
            nc.scalar.activation(e[:, off:], sc_ps[:, off:], EXP)
            if diag:
                if moff == 384:
                    nc.vector.tensor_mul(e[:, 256:512],
                                         e[:, 256:512], tri2[:])
                else:
                    nc.vector.tensor_mul(e[:, moff:moff + 128],
                                         e[:, moff:moff + 128], tri[:])
            pend[(h, kt)] = (e, off)

        def emit_accum(qc, h, kt):
            e, off = pend.pop((h, kt))
            gkv = h // 2
            nkt = 4 * (qc + 1)
            if kt == 0:
                out_ps = outps.tile([128, 512], F32, tag="out")
                den_ps = denps.tile([1, 512], F32, tag="den")
                # +1 phantom logit: seed the PSUM bank, accumulate onto it
                nc.vector.memset(den_ps[:], 1.0)
                hstate[h] = (out_ps, den_ps)
            out_ps, den_ps = hstate[h]
            nc.tensor.matmul(out_ps[:, off:],
                             vnat[kt][:, gkv * 128:(gkv + 1) * 128],
                             e[:, off:],
                             start=(kt == 0), stop=(kt == nkt - 1))
            nc.tensor.matmul(den_ps[:, off:], ones128[:], e[:, off:],
                             start=False, stop=(kt == nkt - 1),
                             skip_group_check=True)
            if kt == nkt - 1:
                # finalize head: oc = out_ps * broadcast(1/den), den seeded +1
                rec = finp.tile([1, 512], F32, tag="rec")
                with nc.allow_low_precision(reason="recip of denom"):
                    nc.vector.reciprocal(rec[:], den_ps[:])
                bcs = finp.tile([128, 512], F32, tag="bcs")
                nc.gpsimd.partition_broadcast(bcs[:], rec[:])
                o = ocp.tile([128, 512], WD, tag=f"oc{h}", name=f"oc{h}_{qc}")
                if WD == F32R:
                    nc.vector.tensor_mul(o[:], out_ps[:], bcs[:])
                else:
                    of = finp.tile([128, 512], F32, tag="ocf")
                    nc.vector.tensor_mul(of[:], out_ps[:], bcs[:])
                    nc.vector.tensor_copy(o[:], of[:])
                oc_cur[h] = o

        wo_o3 = {}

        def emit_wo_chain(oc, qcp, sub, dc, split_dma=False):
            ps3 = w3ps.tile([128, 512], F32, tag="wo3")
            for hf in range(4):
                nc.tensor.matmul(ps3[:], oc[hf][:, ts(sub, 128)],
                                 wo_sb[hf][:, ts(dc, 512)],
                                 start=(hf == 0), stop=(hf == 3))
            st = qcp * 4 + sub
            if dc == 0:
                wo_o3[st] = osbp.tile([128, D], mybir.dt.bfloat16, tag="o3",
                                      name=f"o3_{st}")
            # collect the full [128, D] row block, then one 8KB-per-partition
            # DMA (4x fewer out-write instructions; same descriptor count).
            # split_dma (final q-chunk): write each 512-col piece as soon as
            # its copy lands so the kernel does not drain on a 1MB transfer.
            nc.vector.tensor_copy(wo_o3[st][:, ts(dc, 512)], ps3[:])
            if split_dma:
                nc.sync.dma_start(io["out"][ts(st, 128), ts(dc, 512)],
                                  wo_o3[st][:, ts(dc, 512)])
                if dc == 3:
                    wo_o3.pop(st)
            elif dc == 3:
                nc.sync.dma_start(io["out"][ts(st, 128), :], wo_o3.pop(st)[:])

        CHAINS = [(sub, dc) for sub in range(4) for dc in range(4)]
        for qc in range(NQC):
            items = [(h, kt) for h in range(4) for kt in range(4 * (qc + 1))]
            n = len(items)
            chains = list(CHAINS) if qc > 0 else []
            # front-load the deferred wo chains into the first ~half of this
            # chunk's items so the chunk tail (and final drain) stays clean
            spacing = max(1, n // 32) if chains else 1
            ci = 0
            for i, (h, kt) in enumerate(items):
                emit_scores(qc, h, kt)
                if i >= LAG:
                    emit_accum(qc, *items[i - LAG])
                if chains and ci < len(chains) and i >= LAG \
                        and i % spacing == spacing - 1:
                    emit_wo_chain(oc_prev, qc - 1, *CHAINS[ci])
                    ci += 1
            for i in range(n - LAG, n):
                emit_accum(qc, *items[i])
            while chains and ci < len(chains):
                emit_wo_chain(oc_prev, qc - 1, *CHAINS[ci])
                ci += 1
            oc_prev, oc_cur = oc_cur, [None] * 4
        for sub, dc in CHAINS:
            emit_wo_chain(oc_prev, NQC - 1, sub, dc, split_dma=True)
        if "dbg_qk" in io:
            with tc.tile_pool(name="dbgp", bufs=2) as dbgp:
                for f in range(6):
                    dt_ = dbgp.tile([128, S], F32, tag="dbg")
                    nc.vector.tensor_copy(dt_[:], qkT[f][:])
                    nc.sync.dma_start(io["dbg_qk"][ts(f, 128), :], dt_[:])
                dv = dbgp.tile([128, KF], F32, tag="dbgv")
                nc.vector.tensor_copy(dv[:], vnat[0][:])
                nc.sync.dma_start(io["dbg_v"][:], dv[:])


def _host_prep(x, freqs_cos, freqs_sin, wq, wk, wv, wo):
    """Build the 8 per-core input maps."""
    # de-interleave perm within every 128-col head block: [0,2,..,126,1,3,..,127]
    p128 = np.concatenate([np.arange(0, 128, 2), np.arange(1, 128, 2)])
    permq = np.concatenate([hb * 128 + p128 for hb in range(N_HEADS)])
    permk = np.concatenate([hb * 128 + p128 for hb in range(N_KV_HEADS)])
    wq_p = (wq / np.sqrt(np.float32(HD)))[:, permq]
    wk_p = wk[:, permk]

    cosT = np.ascontiguousarray(freqs_cos.T)            # [64, S]
    sinT = np.ascontiguousarray(freqs_sin.T)
    c2 = np.concatenate([cosT, cosT], 0).astype(np.float32)   # [128, S]
    gtab = np.concatenate([sinT, -sinT], 0).astype(np.float32)

    ii, jj = np.meshgrid(np.arange(128), np.arange(128), indexing="ij")
    tri = (ii <= jj).astype(np.float32)                 # [k, q] allow k<=q

    tri2 = np.concatenate([np.zeros((128, 128), np.float32), tri], 1)
    at, pj, wd = _npdt("attn"), _npdt("proj"), _npdt("wo")
    common = {
        "c2": c2, "g": gtab, "tri": tri.astype(at), "tri2": tri2.astype(at),
        "ones128": np.ones((128, 1), at),
        "idn": np.eye(128, dtype=at),
    }
    in_maps = []
    for core in range(8):
        b, t = divmod(core, TP)
        in_maps.append({
            "xT": np.ascontiguousarray(x[b].T).astype(pj),
            "wq": np.ascontiguousarray(wq_p[:, t * QF:(t + 1) * QF]).astype(pj),
            "wk": np.ascontiguousarray(wk_p[:, t * KF:(t + 1) * KF]).astype(pj),
            "wv": np.ascontiguousarray(wv[:, t * KF:(t + 1) * KF]).astype(pj),
            "wo": np.ascontiguousarray(wo[t * QF:(t + 1) * QF, :]).astype(wd),
            **common,
        })
    return in_maps


def kernel(x, freqs_cos, freqs_sin, wq, wk, wv, wo, _trace=False):
    in_maps = _host_prep(np.asarray(x, np.float32),
                         np.asarray(freqs_cos, np.float32),
                         np.asarray(freqs_sin, np.float32),
                         np.asarray(wq, np.float32), np.asarray(wk, np.float32),
                         np.asarray(wv, np.float32), np.asarray(wo, np.float32))
    if "nc" not in _CACHE:
        _CACHE["nc"] = _build()
    res = bass_utils.run_bass_kernel_spmd(_CACHE["nc"], in_maps, list(range(8)),
                                          trace=_trace)
    _CACHE["last_result"] = res
    out = np.zeros((B, S, D), np.float32)
    for core in range(8):
        b = core // TP
        out[b] += res.results[core]["out"].astype(np.float32)
    return out



# revision 52
# speedup vs baseline: 1.0212x; 1.0212x over previous
"""Trainium2 Bass kernel for nn_Attention (GQA + RoPE + softmax-n + causal).

Full inputs -> shard DP2(batch) x TP4(heads) across 8 cores -> gather+sum.

Per-core device program (all matmuls fp32r, PSUM fp32):
  phase 1: Q^T/K^T/V^T = w.T @ x^T   (x^T streamed in 512-col chunks)
           RoPE on Q^T/K^T via sign-folded tables + DMA partition half-swap
           V^T transposed back to natural V via PE transpose
  phase 2: per q-chunk (512 cols), per head:
           scores^T[k,q] = K^T.T @ Q^T  (causal: N-sliced bands)
           E = exp(scores^T)  (softmax-n: no max subtraction; scores ~N(0,0.8))
           diag 128x128 blocks masked by multiplying a triangle mask
           denom[1,q] = ones.T @ E (+1 phantom logit), accumulated in PSUM
           out^T[hd,q] += V.T @ E ;  out^T *= broadcast(1/denom)
           then output projection for this q-chunk: out += oc.T @ wo_shard

Host: out[b] = sum over 4 TP shards of out_partial.
"""
import sys
import numpy as np

sys.path.insert(0, "/opt/trn_rl_repo")

import concourse.bass as bass
import concourse.bacc as bacc
import concourse.mybir as mybir
import concourse.tile as tile
from concourse import bass_utils
from concourse._compat import with_exitstack

F32 = mybir.dt.float32
F32R = mybir.dt.float32r
EXP = mybir.ActivationFunctionType.Exp
COPY = mybir.ActivationFunctionType.Copy

B, S, D = 2, 2048, 2048
N_HEADS, N_KV_HEADS, HD = 16, 8, 128
TP = 4                      # tensor-parallel ways (x DP2 over batch = 8 cores)
QF = 4 * HD                 # per-core q feature cols   (512)
KF = 2 * HD                 # per-core k/v feature cols (256)
NQT = S // 128              # 16 seq tiles
NQC = S // 512              # 4  q-chunks
ND = D // 128               # 16 contraction tiles
NSC = S // 512              # 4  x^T stream chunks

_CACHE = {}

# matmul dtype config: "f32r" or "bf16" per stage
import os
CFG = {"proj": os.environ.get("K_PROJ", "bf16"),
       "attn": os.environ.get("K_ATTN", "f32r"),
       "wo": os.environ.get("K_WO", "f32r")}


def _dt(stage):
    return F32R if CFG[stage] == "f32r" else mybir.dt.bfloat16


def _npdt(stage):
    import ml_dtypes
    return np.float32 if CFG[stage] == "f32r" else ml_dtypes.bfloat16


def _build(bench_reps=None):
    nc = bacc.Bacc("TRN2", target_bir_lowering=False, debug=False)

    PJ, AT, WD = _dt("proj"), _dt("attn"), _dt("wo")
    names = [("xT", [D, S], PJ), ("wq", [D, QF], PJ), ("wk", [D, KF], PJ),
             ("wv", [D, KF], PJ), ("wo", [QF, D], WD),
             ("c2", [128, S], F32), ("g", [128, S], F32),
             ("tri", [128, 128], AT), ("tri2", [128, 256], AT),
             ("ones128", [128, 1], AT), ("idn", [128, 128], AT)]
    kind = "Internal" if bench_reps else "ExternalInput"
    io = {n: nc.dram_tensor(n, sh, dt, kind=kind) for n, sh, dt in names}
    if bench_reps:
        io["dummy"] = nc.dram_tensor("bench_in", [128, 1], F32,
                                     kind="ExternalInput")
    # bf16 out halves the 16MB/core output traffic; host upcasts+sums
    io["out"] = nc.dram_tensor("out", [S, D], mybir.dt.bfloat16,
                               kind="ExternalOutput")
    if os.environ.get("K_DEBUG"):
        io["dbg_qk"] = nc.dram_tensor("dbg_qk", [6 * 128, S], F32,
                                      kind="ExternalOutput")
        io["dbg_v"] = nc.dram_tensor("dbg_v", [128, KF], F32,
                                     kind="ExternalOutput")

    with tile.TileContext(nc) as tc:
        if bench_reps:
            # fill internal DRAM inputs with benign constants (avoid
            # garbage -> denormal/NaN timing artifacts)
            with tc.tile_pool(name="fillp", bufs=1) as fp:
                f3t = fp.tile([128, 2048], F32, tag="fill32")
                fbt = fp.tile([128, 2048], mybir.dt.bfloat16, tag="fillb")
                nc.gpsimd.memset(f3t[:], 0.001)
                nc.gpsimd.memset(fbt[:], 0.001)
                for n, sh, dt in names:
                    r, c = sh
                    for r0 in range(0, r, 128):
                        rr = min(128, r - r0)
                        for c0 in range(0, c, 2048):
                            cc = min(2048, c - c0)
                            if dt == mybir.dt.bfloat16:
                                srcap = fbt[:rr, :cc]
                            elif dt == F32R:
                                srcap = f3t[:rr, :cc].bitcast(F32R)
                            else:
                                srcap = f3t[:rr, :cc]
                            nc.sync.dma_start(io[n][r0:r0 + rr, c0:c0 + cc],
                                              srcap)
        if bench_reps and bench_reps > 1:
            with tc.For_i(0, bench_reps, 1):
                _emit(tc, nc, io)
        else:
            _emit(tc, nc, io)
    nc.compile()
    return nc


@with_exitstack
def _emit(ctx, tc, nc, io):
    ts = bass.ts
    PJ, AT, WD = _dt("proj"), _dt("attn"), _dt("wo")
    persist = ctx.enter_context(tc.tile_pool(name="persist", bufs=1))

    # ---- persistent SBUF tensors (live whole kernel) ----
    tri = persist.tile([128, 128], AT, tag="tri")
    tri2 = persist.tile([128, 256], AT, tag="tri2")
    ones128 = persist.tile([128, 1], AT, tag="ones128")

    # rotated Q^T/K^T: 6 head tiles [128, S]; V natural: 16 tiles [128, KF]
    qkT = [persist.tile([128, S], AT, tag=f"qkT{f}", name=f"qkT{f}")
           for f in range(6)]
    vnat = [persist.tile([128, KF], AT, tag=f"vnat{st}", name=f"vnat{st}")
            for st in range(NQT)]
    # ================= phase 1: projections + rope + V transpose ==========
    # DMA instruction count is the scarce resource (~630ns HWDGE + ~650ns
    # SEQ per instruction regardless of size), so weights/x^T load as wide
    # merged tiles via rearranged access patterns: wq in 4 DMAs, wk/wv in 2
    # each, x^T in 4 per 512-col chunk, c2/g in 1 each (full S).
    with tc.tile_pool(name="wp", bufs=1) as wp, \
         tc.tile_pool(name="xtp", bufs=2) as xtp, \
         tc.tile_pool(name="rope", bufs=3) as ropep, \
         tc.tile_pool(name="vsb", bufs=2) as vsbp, \
         tc.tile_pool(name="p1ps", bufs=5, space="PSUM") as p1ps, \
         tc.tile_pool(name="vtps", bufs=3, space="PSUM") as vtps:
        idn = wp.tile([128, 128], AT, tag="idn")
        wq_all = wp.tile([128, ND * QF], PJ, tag="wq")      # [128, 8192]
        wk_all = wp.tile([128, ND * KF], PJ, tag="wk")      # [128, 4096]
        wv_all = wp.tile([128, ND * KF], PJ, tag="wv")
        c2_all = wp.tile([128, S], F32, tag="c2")
        g_all = wp.tile([128, S], F32, tag="g")

        def load_xt(sc, t):
            for gq in range(4):
                dst = t[:, gq * 2048:(gq + 1) * 2048].rearrange(
                    "p (d c) -> p d c", d=4)
                src = io["xT"][ts(gq, 512), ts(sc, 512)].rearrange(
                    "(d p) c -> p d c", d=4)
                nc.scalar.dma_start(dst, src)
            return t

        # V chains run first in the f-loop (smallest weight gate: wv 1MB vs
        # wq 2MB), so interleave wv groups with x^T chunk 0; wq lands while
        # the V chains execute.
        xt_tiles = [None] * NSC
        t0 = xtp.tile([128, ND * 512], PJ, tag="xt", name="xt0")

        def wv_grp(d0, gd):
            dst = wv_all[:, d0 * 256:(d0 + gd) * 256].rearrange(
                "p (d c) -> p d c", d=gd)
            src = io["wv"][d0 * 128:(d0 + gd) * 128, :].rearrange(
                "(d p) c -> p d c", d=gd)
            nc.sync.dma_start(dst, src)

        def xt0_grp(d0, gd):
            dstx = t0[:, d0 * 512:(d0 + gd) * 512].rearrange(
                "p (d c) -> p d c", d=gd)
            srcx = io["xT"][d0 * 128:(d0 + gd) * 128, 0:512].rearrange(
                "(d p) c -> p d c", d=gd)
            nc.scalar.dma_start(dstx, srcx)

        wv_grp(0, 2)
        xt0_grp(0, 2)
        wv_grp(2, 2)
        xt0_grp(2, 2)
        wv_grp(4, 4)
        xt0_grp(4, 4)
        wv_grp(8, 8)
        xt0_grp(8, 4)
        xt0_grp(12, 4)
        nc.sync.dma_start(idn[:], io["idn"][:])
        for gq in range(4):
            dst = wq_all[:, gq * 2048:(gq + 1) * 2048].rearrange(
                "p (d c) -> p d c", d=4)
            src = io["wq"][ts(gq, 512), :].rearrange("(d p) c -> p d c", d=4)
            nc.sync.dma_start(dst, src)
        xt_tiles[0] = t0
        add_pend = []
        # c2/g on the scalar queue BEHIND the xt0 groups: their 5.8MB-at-f32
        # transfers must not displace the startup-critical wv/wq/xt0 stream
        # (p1ps=5 gives the first rope ~20us of slack before PE would stall)
        nc.scalar.dma_start(c2_all[:], io["c2"][:])
        nc.scalar.dma_start(g_all[:], io["g"][:])
        for gk in range(2):
            dst = wk_all[:, gk * 2048:(gk + 1) * 2048].rearrange(
                "p (d c) -> p d c", d=8)
            src = io["wk"][ts(gk, 1024), :].rearrange("(d p) c -> p d c", d=8)
            nc.sync.dma_start(dst, src)
        for name, t in [("tri", tri), ("tri2", tri2), ("ones128", ones128)]:
            nc.sync.dma_start(t[:], io[name][:])

        for sc in range(NSC):                    # 512-wide x^T chunks
            cs = ts(sc, 512)
            xt = xt_tiles[sc]
            if sc + 1 < NSC:                     # prefetch next chunk
                xt_tiles[sc + 1] = load_xt(
                    sc + 1, xtp.tile([128, ND * 512], PJ, tag="xt",
                                     name=f"xt{sc + 1}"))
            vts = []
            # f: 0..3 q-heads, 4..5 k-heads, 6..7 v-heads; V first (their
            # weights land first at startup), k last (their rope adds are
            # deferred anyway)
            for idx, f in enumerate((6, 7, 0, 1, 2, 3, 4, 5)):
                if f < 4:
                    wall, wstride, fo = wq_all, QF, f * 128
                elif f < 6:
                    wall, wstride, fo = wk_all, KF, (f - 4) * 128
                else:
                    wall, wstride, fo = wv_all, KF, (f - 6) * 128
                ps = p1ps.tile([128, 512], F32, tag="proj")
                for d in range(ND):
                    nc.tensor.matmul(
                        ps[:],
                        wall[:, d * wstride + fo:d * wstride + fo + 128],
                        xt[:, ts(d, 512)],
                        start=(d == 0), stop=(d == ND - 1))
                # drain V transposes two chains after their V chain (vt copy
                # is certainly done): the PSUM banks and vnat tiles are then
                # released mid-chunk, not in the chunk/phase tail
                if idx >= 2 and vts:
                    fv, vt = vts.pop(0)
                    for sub in range(4):
                        st = sc * 4 + sub
                        tp = vtps.tile([128, 128], AT, tag="vtp")
                        nc.tensor.transpose(tp[:], vt[:, ts(sub, 128)],
                                            idn[:])
                        nc.vector.tensor_copy(
                            vnat[st][:, (fv - 6) * 128:(fv - 5) * 128], tp[:])
                if f < 6:
                    # rope: rot = ps*c2 + halfswap(ps*g)
                    a = ropep.tile([128, 512], F32, tag="ropeA")
                    b = ropep.tile([128, 512], F32, tag="ropeB")
                    bsw = ropep.tile([128, 512], F32, tag="ropeBsw")
                    nc.vector.tensor_mul(a[:], ps[:], c2_all[:, cs])
                    nc.vector.tensor_mul(b[:], ps[:], g_all[:, cs])
                    nc.gpsimd.dma_start(bsw[0:64, :], b[64:128, :])
                    nc.gpsimd.dma_start(bsw[64:128, :], b[0:64, :])
                    if f < 4:
                        nc.vector.tensor_add(qkT[f][:, cs], a[:], bsw[:])
                    else:
                        # defer the k-head adds past the V copies: the add
                        # waits on the swap DMA, and queuing it ahead of the
                        # V copies on DVE stalls the PE transposes at each
                        # sc boundary (the adds aren't needed until phase 2)
                        add_pend.append((f, a, bsw, cs))
                else:
                    # V^T -> copy to SBUF; emitted before the deferred k-head
                    # adds so the PE transposes never queue behind the
                    # swap-DMA-gated rope adds on DVE
                    vt = vsbp.tile([128, 512], AT, tag="vT")
                    nc.vector.tensor_copy(vt[:], ps[:])
                    vts.append((f, vt))
            for f, vt in vts:
                for sub in range(4):
                    st = sc * 4 + sub
                    tp = vtps.tile([128, 128], AT, tag="vtp")
                    nc.tensor.transpose(tp[:], vt[:, ts(sub, 128)], idn[:])
                    nc.vector.tensor_copy(
                        vnat[st][:, (f - 6) * 128:(f - 5) * 128], tp[:])
            for fp, ap_, bswp, csp in add_pend:
                nc.vector.tensor_add(qkT[fp][:, csp], ap_[:], bswp[:])
            add_pend = []

    # ============ phase 2: attention + fused output projection ============
    # Software-pipelined: PE stream = scores(i), scores(i+1), then
    # AV/den(i-LAG) interleaved, so the ACT exp latency never stalls PE.
    # The wo projection for q-chunk qc-1 is interleaved into qc's attention
    # so the recip/broadcast latency at each chunk boundary is hidden.
    with tc.tile_pool(name="ep", bufs=8) as ep, \
         tc.tile_pool(name="ocp", bufs=2) as ocp, \
         tc.tile_pool(name="fin", bufs=2) as finp, \
         tc.tile_pool(name="osb", bufs=3) as osbp, \
         tc.tile_pool(name="scps", bufs=3, space="PSUM") as scps, \
         tc.tile_pool(name="outps", bufs=2, space="PSUM") as outps, \
         tc.tile_pool(name="denps", bufs=1, space="PSUM") as denps, \
         tc.tile_pool(name="w3ps", bufs=2, space="PSUM") as w3ps, \
         tc.tile_pool(name="wop", bufs=1) as wop:
        wo_sb = [wop.tile([128, D], WD, tag=f"wo{hf}", name=f"wo{hf}")
                 for hf in range(4)]
        for hf in range(4):
            nc.sync.dma_start(wo_sb[hf][:], io["wo"][ts(hf, 128), :])

        LAG = 4
        pend = {}          # (h, kt) -> (e tile, off)
        hstate = {}        # h -> (out_ps, den_ps)
        oc_cur = [None] * 4
        oc_prev = None

        def emit_scores(qc, h, kt):
            qs = qc * 512
            gkv = h // 2
            qT, kT = qkT[h], qkT[4 + gkv]
            off = max(0, 128 * kt - qs)
            diag = kt >= 4 * qc
            moff = off
            if off == 384:
                off = 256        # keep N>=256 (fp32r full rate)
            sc_ps = scps.tile([128, 512], F32, tag="sc")
            nc.tensor.matmul(sc_ps[:, off:], kT[:, ts(kt, 128)],
                             qT[:, qs + off:qs + 512],
                             start=True, stop=True)
            e = ep.tile([128, 512], AT, tag="e")
            nc.scalar.activation(e[:, off:], sc_ps[:, off:], EXP)
            if diag:
                if moff == 384:
                    nc.vector.tensor_mul(e[:, 256:512],
                                         e[:, 256:512], tri2[:])
                else:
                    nc.vector.tensor_mul(e[:, moff:moff + 128],
                                         e[:, moff:moff + 128], tri[:])
            pend[(h, kt)] = (e, off)

        def emit_accum(qc, h, kt):
            e, off = pend.pop((h, kt))
            gkv = h // 2
            nkt = 4 * (qc + 1)
            if kt == 0:
                out_ps = outps.tile([128, 512], F32, tag="out")
                den_ps = denps.tile([1, 512], F32, tag="den")
                # +1 phantom logit: seed the PSUM bank, accumulate onto it
                nc.vector.memset(den_ps[:], 1.0)
                hstate[h] = (out_ps, den_ps)
            out_ps, den_ps = hstate[h]
            nc.tensor.matmul(out_ps[:, off:],
                             vnat[kt][:, gkv * 128:(gkv + 1) * 128],
                             e[:, off:],
                             start=(kt == 0), stop=(kt == nkt - 1))
            nc.tensor.matmul(den_ps[:, off:], ones128[:], e[:, off:],
                             start=False, stop=(kt == nkt - 1),
                             skip_group_check=True)
            if kt == nkt - 1:
                # finalize head: oc = out_ps * broadcast(1/den), den seeded +1
                rec = finp.tile([1, 512], F32, tag="rec")
                with nc.allow_low_precision(reason="recip of denom"):
                    nc.vector.reciprocal(rec[:], den_ps[:])
                bcs = finp.tile([128, 512], F32, tag="bcs")
                nc.gpsimd.partition_broadcast(bcs[:], rec[:])
                o = ocp.tile([128, 512], WD, tag=f"oc{h}", name=f"oc{h}_{qc}")
                if WD == F32R:
                    nc.vector.tensor_mul(o[:], out_ps[:], bcs[:])
                else:
                    of = finp.tile([128, 512], F32, tag="ocf")
                    nc.vector.tensor_mul(of[:], out_ps[:], bcs[:])
                    nc.vector.tensor_copy(o[:], of[:])
                oc_cur[h] = o

        wo_o3 = {}

        def emit_wo_chain(oc, qcp, sub, dc, split_dma=False):
            ps3 = w3ps.tile([128, 512], F32, tag="wo3")
            for hf in range(4):
                nc.tensor.matmul(ps3[:], oc[hf][:, ts(sub, 128)],
                                 wo_sb[hf][:, ts(dc, 512)],
                                 start=(hf == 0), stop=(hf == 3))
            st = qcp * 4 + sub
            if dc == 0:
                wo_o3[st] = osbp.tile([128, D], mybir.dt.bfloat16, tag="o3",
                                      name=f"o3_{st}")
            # collect the full [128, D] row block, then one 8KB-per-partition
            # DMA (4x fewer out-write instructions; same descriptor count).
            # split_dma (final q-chunk): write each 512-col piece as soon as
            # its copy lands so the kernel does not drain on a 1MB transfer.
            nc.vector.tensor_copy(wo_o3[st][:, ts(dc, 512)], ps3[:])
            if split_dma:
                nc.sync.dma_start(io["out"][ts(st, 128), ts(dc, 512)],
                                  wo_o3[st][:, ts(dc, 512)])
                if dc == 3:
                    wo_o3.pop(st)
            elif dc == 3:
                nc.sync.dma_start(io["out"][ts(st, 128), :], wo_o3.pop(st)[:])

        CHAINS = [(sub, dc) for sub in range(4) for dc in range(4)]
        for qc in range(NQC):
            items = [(h, kt) for h in range(4) for kt in range(4 * (qc + 1))]
            n = len(items)
            chains = list(CHAINS) if qc > 0 else []
            # front-load the deferred wo chains into the first ~half of this
            # chunk's items so the chunk tail (and final drain) stays clean
            spacing = max(1, n // 32) if chains else 1
            ci = 0
            for i, (h, kt) in enumerate(items):
                emit_scores(qc, h, kt)
                if i >= LAG:
                    emit_accum(qc, *items[i - LAG])
                if chains and ci < len(chains) and i >= LAG \
                        and i % spacing == spacing - 1:
                    emit_wo_chain(oc_prev, qc - 1, *CHAINS[ci])
                    ci += 1
            for i in range(n - LAG, n):
                emit_accum(qc, *items[i])
            while chains and ci < len(chains):
                emit_wo_chain(oc_prev, qc - 1, *CHAINS[ci])
                ci += 1
            oc_prev, oc_cur = oc_cur, [None] * 4
        for sub, dc in CHAINS:
            emit_wo_chain(oc_prev, NQC - 1, sub, dc, split_dma=True)
        if "dbg_qk" in io:
            with tc.tile_pool(name="dbgp", bufs=2) as dbgp:
                for f in range(6):
                    dt_ = dbgp.tile([128, S], F32, tag="dbg")
                    nc.vector.tensor_copy(dt_[:], qkT[f][:])
                    nc.sync.dma_start(io["dbg_qk"][ts(f, 128), :], dt_[:])
                dv = dbgp.tile([128, KF], F32, tag="dbgv")
                nc.vector.tensor_copy(dv[:], vnat[0][:])
                nc.sync.dma_start(io["dbg_v"][:], dv[:])


def _host_prep(x, freqs_cos, freqs_sin, wq, wk, wv, wo):
    """Build the 8 per-core input maps."""
    # de-interleave perm within every 128-col head block: [0,2,..,126,1,3,..,127]
    p128 = np.concatenate([np.arange(0, 128, 2), np.arange(1, 128, 2)])
    permq = np.concatenate([hb * 128 + p128 for hb in range(N_HEADS)])
    permk = np.concatenate([hb * 128 + p128 for hb in range(N_KV_HEADS)])
    wq_p = (wq / np.sqrt(np.float32(HD)))[:, permq]
    wk_p = wk[:, permk]

    cosT = np.ascontiguousarray(freqs_cos.T)            # [64, S]
    sinT = np.ascontiguousarray(freqs_sin.T)
    c2 = np.concatenate([cosT, cosT], 0).astype(np.float32)   # [128, S]
    gtab = np.concatenate([sinT, -sinT], 0).astype(np.float32)

    ii, jj = np.meshgrid(np.arange(128), np.arange(128), indexing="ij")
    tri = (ii <= jj).astype(np.float32)                 # [k, q] allow k<=q

    tri2 = np.concatenate([np.zeros((128, 128), np.float32), tri], 1)
    at, pj, wd = _npdt("attn"), _npdt("proj"), _npdt("wo")
    common = {
        "c2": c2, "g": gtab, "tri": tri.astype(at), "tri2": tri2.astype(at),
        "ones128": np.ones((128, 1), at),
        "idn": np.eye(128, dtype=at),
    }
    in_maps = []
    for core in range(8):
        b, t = divmod(core, TP)
        in_maps.append({
            "xT": np.ascontiguousarray(x[b].T).astype(pj),
            "wq": np.ascontiguousarray(wq_p[:, t * QF:(t + 1) * QF]).astype(pj),
            "wk": np.ascontiguousarray(wk_p[:, t * KF:(t + 1) * KF]).astype(pj),
            "wv": np.ascontiguousarray(wv[:, t * KF:(t + 1) * KF]).astype(pj),
            "wo": np.ascontiguousarray(wo[t * QF:(t + 1) * QF, :]).astype(wd),
            **common,
        })
    return in_maps


def kernel(x, freqs_cos, freqs_sin, wq, wk, wv, wo, _trace=False):
    in_maps = _host_prep(np.asarray(x, np.float32),
                         np.asarray(freqs_cos, np.float32),
                         np.asarray(freqs_sin, np.float32),
                         np.asarray(wq, np.float32), np.asarray(wk, np.float32),
                         np.asarray(wv, np.float32), np.asarray(wo, np.float32))
    if "nc" not in _CACHE:
        _CACHE["nc"] = _build()
    res = bass_utils.run_bass_kernel_spmd(_CACHE["nc"], in_maps, list(range(8)),
                                          trace=_trace)
    _CACHE["last_result"] = res
    out = np.zeros((B, S, D), np.float32)
    for core in range(8):
        b = core // TP
        out[b] += res.results[core]["out"].astype(np.float32)
    return out

